# revision 2
# baseline (speedup 1.0000x reference)
"""Trainium2 Bass kernel for BasicLSTM (B=64, T=512, D=U=512).

Sharding: data-parallel over batch across 8 cores (8 rows/core), weights
replicated; the sequential time scan runs locally per core.

Per-core strategy (everything unit-major / "transposed", all-SBUF):
  Phase A: zx.T = Wk.T @ x.T + b computed directly in unit-major layout.
    x is loaded with fast contiguous DMAs, converted to bf16, transposed
    on-chip via the DMA xbar (dedicated queue), then used as the moving
    operand against stationary bf16 Wk tiles.  Bias is applied via the ACT
    per-partition bias during PSUM->SBUF copy-out.  The whole
    zx.T [128p, T*(16m*8b)] stays resident in SBUF as bf16 (16 MB).
  Phase B: 512-step scan with zero DMA.  Gate m-tiles are reordered
    [i,f,o,g] and the 16 m-tiles are processed in two halves, each into its
    own PSUM bank, so the elementwise tail of the first half overlaps the
    matmuls of the second:
      z.T[t] = sum_k Wr[k,m].T @ h.T[k]      (PE, bf16+FWL, 64 LDW+MM)
      psum += zx.T[t]                        (DVE, in place)
      i,f,o = sigmoid(psum), g = tanh(psum)  (ACT, reads PSUM)
      c' = f*c + i*g ; h' = o*tanh(c')       (DVE/ACT)
    h/c are split into per-half tiles; h is bf16 (feeds the next matmul),
    c stays fp32.  The final h is computed in fp32 and DMA'd out.
"""

import numpy as np

B, T, D, U = 64, 512, 512, 512
G = 4 * U            # gates
P = 128              # partitions
N_CORES = 8
B_LOC = B // N_CORES  # 8
KD = D // P          # 4 k-tiles for x@Wk
KU = U // P          # 4 k-tiles for h@Wr
M = G // P           # 16 m-tiles of gates
TC = 64              # timesteps per phase-A chunk
FB = M * B_LOC       # 128 free cols of z per step
HB = FB // 2         # 64 cols per half

# gate reordering: new m-tile order [i, f, o, g] -> original m-tile index
PERMM = list(range(8)) + [12, 13, 14, 15] + [8, 9, 10, 11]
# halves: half h holds m-tiles {4a + q : a in 0..3} for q in {2h, 2h+1}
HALF_MS = [[0, 4, 8, 12, 1, 5, 9, 13], [2, 6, 10, 14, 3, 7, 11, 15]]

_CACHE = {}


def _build(time_steps=T):
    import concourse.bacc as bacc
    import concourse.tile as tile
    import concourse.mybir as mybir
    from bass_rust import add_dep_helper

    f32 = mybir.dt.float32
    bf16 = mybir.dt.bfloat16
    AF = mybir.ActivationFunctionType

    nc = bacc.Bacc(
        "TRN2",
        target_bir_lowering=False,
        debug=False,
        enable_asserts=True,
        num_devices=N_CORES,
    )

    x_h = nc.dram_tensor("x", [B_LOC, T, D], f32, kind="ExternalInput")
    wk_h = nc.dram_tensor("Wk", [D, G], f32, kind="ExternalInput")
    wr_h = nc.dram_tensor("Wr", [U, G], f32, kind="ExternalInput")
    b_h = nc.dram_tensor("b", [G], f32, kind="ExternalInput")
    out_h = nc.dram_tensor("h_last", [B_LOC, U], f32, kind="ExternalOutput")

    x_ap = x_h.ap()

    def load_weight_bf16(dst, src_h, stage_pool):
        """[512, 2048] fp32 weight -> dst bf16 [128, 64*128] laid out as
        (k, new_m) tiles of [128, 128] with the [i,f,o,g] gate reorder."""
        for k in range(KD):
            st = stage_pool.tile([P, G], f32, name="wstage", tag="wstage")
            nc.gpsimd.dma_start(st[:], src_h.ap()[k * P:(k + 1) * P, :])
            for nm0, om0, w in ((0, 0, 8), (8, 12, 4), (12, 8, 4)):
                nc.vector.tensor_copy(
                    dst[:, (k * M + nm0) * P:(k * M + nm0 + w) * P],
                    st[:, om0 * P:(om0 + w) * P],
                )

    with tile.TileContext(nc) as tc:
        with (
            tc.tile_pool(name="persist", bufs=1) as persist_pool,
        ):
            # zx.T resident in SBUF: col = m*(T*8) + b*64 + t  (bf16, 128KB/par)
            # (phase A writes [128, 512] contiguous per (m, chunk); the scan
            #  reads a strided comb per step, which is free on DVE)
            zxT = persist_pool.tile([P, T * FB], bf16)
            zxT4 = zxT.rearrange("p (m b t) -> p m b t", m=M, b=B_LOC)
            b_sb = persist_pool.tile([P, M], f32)
            nc.sync.dma_start(b_sb[:], b_h.ap().rearrange("(m p) -> p m", p=P))

            # ---------------- Phase A: zx.T = Wk.T @ x.T + b ----------------
            with (
                tc.tile_pool(name="wk", bufs=1) as wk_pool,
                tc.tile_pool(name="stage", bufs=2) as stage_pool,
                tc.tile_pool(name="nat", bufs=2) as nat_pool,
                tc.tile_pool(name="xtb", bufs=2) as xtb_pool,
                tc.tile_pool(name="gemm_psum", bufs=4, space="PSUM") as gps_pool,
            ):
                wk_sb = wk_pool.tile([P, KD * G], bf16)
                load_weight_bf16(wk_sb, wk_h, stage_pool)

                for chunk in range(T // TC):
                    t0 = chunk * TC
                    # natural x loads: tile bp holds rows (b=2bp..2bp+1, t0..t0+63)
                    natbs = []
                    for bp in range(4):
                        nat = nat_pool.tile([P, D], f32, name="nat", tag=f"nat{bp}")
                        for j in range(2):
                            nc.gpsimd.dma_start(
                                nat[j * TC:(j + 1) * TC, :],
                                x_ap[2 * bp + j, t0:t0 + TC, :],
                            )
                        natb = nat_pool.tile([P, D], bf16, name="natb", tag=f"natb{bp}")
                        nc.vector.tensor_copy(natb[:], nat[:])
                        natbs.append(natb)
                    # xbar transposes: xtb[k] cols = b*64 + t  (b-major)
                    xtbs = []
                    for k in range(KD):
                        xtb = xtb_pool.tile([P, TC * B_LOC], bf16,
                                            name=f"xtb{k}", tag=f"xtb{k}")
                        for bp in range(4):
                            nc.sync.dma_start(
                                xtb[:, bp * P:(bp + 1) * P],
                                natbs[bp][:, k * P:(k + 1) * P],
                                transpose=True,
                            )
                        xtbs.append(xtb)
                    for m in range(M):
                        ps = gps_pool.tile([P, TC * B_LOC], f32,
                                           name="gps", tag="gps")
                        for k in range(KD):
                            nc.tensor.matmul(
                                ps[:],
                                wk_sb[:, (k * M + m) * P:(k * M + m + 1) * P],
                                xtbs[k][:],
                                start=(k == 0),
                                stop=(k == KD - 1),
                            )
                        # copy-out + per-partition bias
                        # psum free = (b, t) b-major = contiguous dst slice
                        nc.scalar.activation(
                            zxT4[:, m, :, t0:t0 + TC],
                            ps.rearrange("p (b t) -> p b t", t=TC)[:],
                            AF.Identity,
                            bias=b_sb[:, PERMM[m]:PERMM[m] + 1],
                        )

            # ---------------- Phase B: the scan ----------------
            with (
                tc.tile_pool(name="wr", bufs=1) as wr_pool,
                tc.tile_pool(name="wstage2", bufs=2) as wstage2_pool,
                tc.tile_pool(name="state", bufs=1) as st_pool,
                tc.tile_pool(name="gates", bufs=2) as gate_pool,
                tc.tile_pool(name="tmp", bufs=2) as tmp_pool,
                tc.tile_pool(name="scan_psum", bufs=2, space="PSUM") as sps_pool,
            ):
                wr_sb = wr_pool.tile([P, KU * G], bf16)
                load_weight_bf16(wr_sb, wr_h, wstage2_pool)

                # h: bf16 per (parity, half); c: fp32 per (parity, half)
                hs = [[st_pool.tile([P, 2 * B_LOC], bf16, name=f"h{i}{j}")
                       for j in range(2)] for i in range(2)]
                cs = [[st_pool.tile([P, 2 * B_LOC], f32, name=f"c{i}{j}")
                       for j in range(2)] for i in range(2)]
                for j in range(2):
                    nc.vector.memset(hs[0][j][:], 0.0)
                    nc.vector.memset(cs[0][j][:], 0.0)
                hf = st_pool.tile([P, KU * B_LOC], f32, name="hf")

                # psum half tile col layout: a*16 + q*8 + b, a = gate class
                for t in range(time_steps):
                    pp = t % 2
                    qq = 1 - pp
                    h_prev = hs[pp]
                    pss = [sps_pool.tile([P, HB], f32, name=f"ps{hf_}",
                                         tag=f"ps{hf_}") for hf_ in range(2)]
                    # MM order: [half0 kk{0,1}], [half0 kk{2,3}],
                    #           [half1 kk{0,1}], [half1 kk{2,3}]
                    # - the first 16 pairs only need h half 0 (overlap with the
                    #   previous step's half-1 tail)
                    # - ps0 is complete after 32 pairs, so its tail starts at
                    #   the PE block's midpoint
                    # PSUM accumulation relies on per-element has_written:
                    # start=True only on the first MM per bank.
                    for half in range(2):
                        firstmm = True
                        for kpair in range(2):
                            for m in HALF_MS[half]:
                                a, q = m // 4, m % 4 - 2 * half
                                dst = pss[half][:, a * 16 + q * 8:
                                                a * 16 + q * 8 + 8]
                                for kk in (2 * kpair, 2 * kpair + 1):
                                    nc.tensor.matmul(
                                        dst,
                                        wr_sb[:, (kk * M + m) * P:
                                              (kk * M + m + 1) * P],
                                        h_prev[kk // 2][:, (kk % 2) * B_LOC:
                                                        (kk % 2 + 1) * B_LOC],
                                        start=firstmm,
                                        stop=(kpair == 1 and kk == KU - 1
                                              and m == HALF_MS[half][-1]),
                                        skip_group_check=True,
                                    )
                                    firstmm = False
                    last = t == time_steps - 1
                    prev_tc = None
                    prev_hmul = None
                    for half in range(2):
                        ps = pss[half]
                        # zx comb for this half: m = 4a + q + 2*half, all b,
                        # one t element each
                        zxh = (zxT4
                               .rearrange("p (a qq) b t -> p a qq b t", qq=4)
                               [:, :, 2 * half:2 * half + 2, :, t])
                        ps4 = ps.rearrange("p (a q b) -> p a q b", q=2, b=B_LOC)
                        i_zadd = nc.vector.tensor_add(ps4[:], ps4[:], zxh)
                        gt = gate_pool.tile([P, HB], f32, name=f"gt{half}",
                                            tag=f"gt{half}")
                        i_sig = nc.scalar.activation(gt[:, 0:48], ps[:, 0:48],
                                                     AF.Sigmoid)
                        nc.scalar.activation(gt[:, 48:64], ps[:, 48:64], AF.Tanh)
                        t1 = tmp_pool.tile([P, 2 * B_LOC], f32,
                                           name=f"t1{half}", tag=f"t1{half}")
                        nc.vector.tensor_mul(t1[:], gt[:, 16:32], cs[pp][half][:])
                        t2 = tmp_pool.tile([P, 2 * B_LOC], f32,
                                           name=f"t2{half}", tag=f"t2{half}")
                        nc.vector.tensor_mul(t2[:], gt[:, 0:16], gt[:, 48:64])
                        nc.vector.tensor_add(cs[qq][half][:], t1[:], t2[:])
                        tc_t = tmp_pool.tile([P, 2 * B_LOC], f32,
                                             name=f"tc{half}", tag=f"tc{half}")
                        i_tc = nc.scalar.activation(tc_t[:], cs[qq][half][:],
                                                    AF.Tanh)
                        if last:
                            i_hmul = nc.vector.tensor_mul(
                                hf[:, half * 16:(half + 1) * 16],
                                gt[:, 32:48], tc_t[:],
                            )
                        else:
                            i_hmul = nc.vector.tensor_mul(hs[qq][half][:],
                                                          gt[:, 32:48], tc_t[:])
                        if half == 1 and prev_tc is not None:
                            # keep ACT/DVE focused on the half-0 chain: half-1
                            # tail slots in only once half 0's h is produced
                            add_dep_helper(i_sig.ins, prev_tc.ins,
                                           reason="tail1 ACT after tail0 tanh_c")
                            add_dep_helper(i_zadd.ins, prev_hmul.ins,
                                           reason="tail1 zadd after tail0 h")
                        prev_tc, prev_hmul = i_tc, i_hmul

                for kk in range(KU):
                    nc.sync.dma_start(
                        out_h.ap()[:, kk * P:(kk + 1) * P].rearrange("b p -> p b"),
                        hf[:, kk * B_LOC:(kk + 1) * B_LOC],
                    )

    nc.compile()
    return nc


def _get_nc(time_steps=T):
    key = time_steps
    if key not in _CACHE:
        _CACHE[key] = _build(time_steps)
    return _CACHE[key]


def _make_in_maps(inputs):
    x = np.ascontiguousarray(np.asarray(inputs["x"], dtype=np.float32))
    Wk = np.ascontiguousarray(np.asarray(inputs["Wk"], dtype=np.float32))
    Wr = np.ascontiguousarray(np.asarray(inputs["Wr"], dtype=np.float32))
    b = np.ascontiguousarray(np.asarray(inputs["b"], dtype=np.float32))
    return [
        {
            "x": x[c * B_LOC:(c + 1) * B_LOC],
            "Wk": Wk,
            "Wr": Wr,
            "b": b,
        }
        for c in range(N_CORES)
    ]


def _gather_output(res):
    return np.concatenate(
        [res.results[c]["h_last"] for c in range(N_CORES)], axis=0
    )


def kernel(x, Wk, Wr, b):
    from concourse import bass_utils

    nc = _get_nc(T)
    in_maps = _make_in_maps({"x": x, "Wk": Wk, "Wr": Wr, "b": b})
    res = bass_utils.run_bass_kernel_spmd(nc, in_maps, core_ids=list(range(N_CORES)))
    return _gather_output(res)



# revision 31
# speedup vs baseline: 5.3174x; 5.3174x over previous
"""Trainium2 Bass kernel for BasicLSTM (B=64, T=512, D=U=512).

Sharding: data-parallel over batch across 8 cores (8 rows/core), weights
replicated; the sequential time scan runs locally per core.

Per-core strategy (unit-major / "transposed", all-SBUF), fully interleaved:

  The input projection zx.T = Wk.T @ x.T + b (phase A) is pipelined INTO the
  scan: chunk c's x-loads/casts/xbar-transposes run during scan steps of
  chunk c-2, its GEMM matmuls (1 per scan step, N=512) run during chunk c-1's
  steps, landing in the PE's tail-wait window.  Only chunk 0 (+ chunk 1 input)
  runs as a prefix.

  The scan computes, per step, z.T[t] in two half-PSUM groups (8 m-tiles of
  [i,f,o,g] each).  Each group = 1 identity-matmul (accumulates zx.T[t] comb
  into PSUM -- no DVE zadd) + 32 LDW+MM pairs (bf16 Wr tiles stationary,
  h.T moving, N=8).

  The elementwise tail uses tanh(x) = 2*sigmoid(2x) - 1 with all scales
  pre-folded into weights: Wk/Wr/b gate-g columns x2, Wr globally x2; h is
  stored as h/2 (bf16) and c as C=2c (fp32).  Per half:
      s   = sigmoid(psum)              (ACT, 64 cols; g-cols hold 2*z_g)
      v   = (s_g - 0.5) * s_i          (DVE scalar_tensor_tensor)
      C'  = 4*v + f*C                  (DVE mul + affine_then_add)
      sc  = sigmoid(C')                (ACT; tanh(c') = 2*sc - 1)
      h'  = (sc - 0.5) * s_o           (DVE STT -> bf16, = h/2)
  Chain: sigma -> STT -> AFF -> sigma_c -> STT, 2 ACT + 3-4 DVE ops/half.
  The final h is scaled back by 2 and DMA'd out as fp32.
"""

import numpy as np

B, T, D, U = 64, 512, 512, 512
G = 4 * U            # gates
P = 128              # partitions
N_CORES = 8
B_LOC = B // N_CORES  # 8
KD = D // P          # 4 k-tiles for x@Wk
KU = U // P          # 4 k-tiles for h@Wr
M = G // P           # 16 m-tiles of gates
TC = 64              # timesteps per phase-A chunk
NCH = T // TC        # 8 chunks
FB = M * B_LOC       # 128 free cols of z per step
HB = FB // 2         # 64 cols per half

# gate reordering: new m-tile order [i, f, o, g] -> original m-tile index
PERMM = list(range(8)) + [12, 13, 14, 15] + [8, 9, 10, 11]
# half h holds m-tiles m = 4a + q + 2h for q in {0,1}; units 256h..256h+256
HALF_MS = [[4 * a + q + 2 * h for q in (0, 1) for a in range(4)] for h in range(2)]

N_DUMMY = 0          # warm-fill dummy matmuls per scan step (HAM can't be
                     # kept warm at this duty cycle; kept for experiments)

_CACHE = {}


def _build(time_steps=T):
    import concourse.bacc as bacc
    import concourse.tile as tile
    import concourse.mybir as mybir
    from bass_rust import add_dep_helper

    f32 = mybir.dt.float32
    bf16 = mybir.dt.bfloat16
    fp8 = mybir.dt.float8e3
    AF = mybir.ActivationFunctionType
    ALU = mybir.AluOpType

    nc = bacc.Bacc(
        "TRN2",
        target_bir_lowering=False,
        debug=False,
        enable_asserts=True,
        num_devices=N_CORES,
    )

    x_h = nc.dram_tensor("x", [B_LOC, T, D], f32, kind="ExternalInput")
    wk_h = nc.dram_tensor("Wk", [D, G], f32, kind="ExternalInput")
    wr_h = nc.dram_tensor("Wr", [U, G], f32, kind="ExternalInput")
    b_h = nc.dram_tensor("b", [G], f32, kind="ExternalInput")
    out_h = nc.dram_tensor("h_last", [B_LOC, U], f32, kind="ExternalOutput")

    x_ap = x_h.ap()

    with tile.TileContext(nc) as tc:
        with (
            tc.tile_pool(name="persist", bufs=1) as persist_pool,
            tc.tile_pool(name="stage", bufs=2) as stage_pool,
            tc.tile_pool(name="nat", bufs=1) as nat_pool,
            tc.tile_pool(name="xtb", bufs=2) as xtb_pool,
            tc.tile_pool(name="state", bufs=1) as st_pool,
            tc.tile_pool(name="gates", bufs=2) as gate_pool,
            tc.tile_pool(name="tmp", bufs=2) as tmp_pool,
            tc.tile_pool(name="apsum", bufs=2, space="PSUM") as aps_pool,
            tc.tile_pool(name="spsum", bufs=2, space="PSUM") as sps_pool,
            tc.tile_pool(name="dpsum", bufs=1, space="PSUM") as dps_pool,
        ):
            # ---------------- persistent SBUF ----------------
            # zx.T: col = m*(B_LOC*T) + b*T + t   (bf16, 128KB/partition)
            zxT = persist_pool.tile([P, T * FB], bf16)
            zxT4 = zxT.rearrange("p (m b t) -> p m b t", m=M, b=B_LOC)
            b_sb = persist_pool.tile([P, M], f32)
            wk_sb = persist_pool.tile([P, KD * G], bf16)
            wr_sb = persist_pool.tile([P, KU * G], bf16)
            ident = persist_pool.tile([P, P], bf16)

            nc.sync.dma_start(b_sb[:], b_h.ap().rearrange("(m p) -> p m", p=P))
            # fold tanh->sigmoid x2 into the g-gate bias (orig m 8..11)
            nc.vector.tensor_scalar_mul(b_sb[:, 8:12], b_sb[:, 8:12], 2.0)

            # identity (bf16) for the zx-accumulate matmuls
            nc.vector.memset(ident[:], 1.0)
            nc.gpsimd.affine_select(
                ident[:], ident[:], pattern=[[-1, P]],
                compare_op=ALU.is_equal, fill=0.0,
                base=0, channel_multiplier=1,
            )

            def load_weight(dst, src_h, base_scale):
                """fp32 [512, 2048] -> dst (k, new_m) tiles with the
                [i,f,o,g] reorder; g-tiles get base_scale*2."""
                for k in range(KD):
                    st = stage_pool.tile([P, G], f32, name="wstage", tag="wstage")
                    nc.sync.dma_start(st[:], src_h.ap()[k * P:(k + 1) * P, :])
                    for nm0, om0, w, sc in (
                        (0, 0, 8, base_scale),          # i, f
                        (8, 12, 4, base_scale),         # o
                        (12, 8, 4, base_scale * 2.0),   # g (tanh->sigmoid)
                    ):
                        d = dst[:, (k * M + nm0) * P:(k * M + nm0 + w) * P]
                        s = st[:, om0 * P:(om0 + w) * P]
                        if sc == 1.0:
                            nc.vector.tensor_copy(d, s)
                        else:
                            nc.vector.tensor_scalar_mul(d, s, sc)

            load_weight(wk_sb, wk_h, 1.0)

            # ---------------- state ----------------
            # h (bf16, = true h / 2): [parity][half] each [P, 2*B_LOC]
            hs = [[st_pool.tile([P, 2 * B_LOC], bf16, name=f"h{i}{j}")
                   for j in range(2)] for i in range(2)]
            # C = 2c (fp32), in-place updated (DVE-ordered)
            cs = [st_pool.tile([P, 2 * B_LOC], f32, name=f"c{j}") for j in range(2)]
            for j in range(2):
                nc.vector.memset(hs[0][j][:], 0.0)
                nc.vector.memset(cs[j][:], 0.0)
            hf = st_pool.tile([P, KU * B_LOC], f32, name="hf")
            # scratch PSUM bank for HAM warm-keeping dummy matmuls
            dps = dps_pool.tile([P, HB], f32, name="dps")

            # ---------------- phase-A helpers ----------------
            # per-chunk tile objects (ring slots rotate per chunk; slices of
            # one logical buffer must come from ONE .tile() allocation)
            chunk_tiles = {}
            pend_aps = {}

            def _ctiles(c):
                if c not in chunk_tiles:
                    chunk_tiles[c] = {
                        "nat": [nat_pool.tile([P, D], f32, name="nat",
                                              tag=f"nat{bp}") for bp in range(4)],
                        "natb": [nat_pool.tile([P, D], bf16, name="natb",
                                               tag=f"natb{bp}") for bp in range(4)],
                        "xtb": [xtb_pool.tile([P, TC * B_LOC], bf16,
                                              name=f"xtb{k}", tag=f"xtb{k}")
                                for k in range(KD)],
                    }
                return chunk_tiles[c]

            def chunk_input(c, j):
                """Input pipeline work-item j (0..63) for chunk c: 8 x-row
                DMAs, 4 casts, 16 xbar transposes."""
                t0 = c * TC
                ct = _ctiles(c)
                if j < 8:
                    bp, jj = j // 2, j % 2
                    nc.gpsimd.dma_start(
                        ct["nat"][bp][jj * TC:(jj + 1) * TC, :],
                        x_ap[2 * bp + jj, t0:t0 + TC, :],
                    )
                elif j in (8, 12, 16, 20):
                    bp = (j - 8) // 4
                    nc.gpsimd.tensor_copy(ct["natb"][bp][:], ct["nat"][bp][:])
                elif 24 <= j < 56 and (j - 24) % 2 == 0:
                    tr = (j - 24) // 2
                    k, bp = tr // 4, tr % 4
                    nc.sync.dma_start(
                        ct["xtb"][k][:, bp * P:(bp + 1) * P],
                        ct["natb"][bp][:, k * P:(k + 1) * P],
                        transpose=True,
                    )

            def chunk_mm(c, j, pin_mm=None, pin_act=None):
                """GEMM work-item j (0..63) for chunk c: one N=512 matmul;
                every 4th completes an m-group -> ACT copy-out with bias.
                pin_mm/pin_act: instructions this item is ordered after, to
                slot it into a scan step's idle window."""
                t0 = c * TC
                ct = _ctiles(c)
                m_a, k_a = j // 4, j % 4
                if k_a == 0:
                    pend_aps[c] = aps_pool.tile([P, TC * B_LOC], f32,
                                                name="aps", tag="aps")
                aps = pend_aps[c]
                i_mm = nc.tensor.matmul(
                    aps[:],
                    wk_sb[:, (k_a * M + m_a) * P:(k_a * M + m_a + 1) * P],
                    ct["xtb"][k_a][:],
                    start=(k_a == 0),
                    stop=(k_a == KD - 1),
                    skip_group_check=True,
                )
                if pin_mm is not None:
                    add_dep_helper(i_mm.ins, pin_mm.ins,
                                   reason="A-MM in scan tail window")
                if k_a == KD - 1:
                    i_act = nc.scalar.activation(
                        zxT4[:, m_a, :, t0:t0 + TC],
                        aps.rearrange("p (b t) -> p b t", t=TC)[:],
                        AF.Identity,
                        bias=b_sb[:, PERMM[m_a]:PERMM[m_a] + 1],
                    )
                    if pin_act is not None:
                        add_dep_helper(i_act.ins, pin_act.ins,
                                       reason="A-copyout after step sigmoids")

            # ---------------- prefix ----------------
            for j in range(TC):
                chunk_input(0, j)
            for j in range(TC):
                chunk_mm(0, j)
            # Wr staging overlaps chunk0's GEMMs (sync-DMA + DVE vs PE)
            load_weight(wr_sb, wr_h, 2.0)   # x2: h is stored as h/2
            for j in range(TC):
                chunk_input(1, j)

            # ---------------- the scan ----------------
            for t in range(time_steps):
                pp = t % 2
                qq = 1 - pp
                h_prev = hs[pp]
                # --- PE: two half psum groups.  Pair blocks are dep-chained
                # into the LP-optimal order [Akk01(16), B01a(8), Akk23(16,
                # stop A), B01b(8), Bkk23(16, stop B)]: half0's psum closes
                # after 40 pairs while half1's h' still lands just in time
                # for the next step's kk23 pairs. ---
                pss = []
                for half in range(2):
                    ps = sps_pool.tile([P, HB], f32, name=f"ps{half}",
                                       tag=f"ps{half}")
                    pss.append(ps)
                    zxh = (zxT4
                           .rearrange("p (a qq) b t -> p a qq b t", qq=4)
                           [:, :, 2 * half:2 * half + 2, :, t])
                    nc.tensor.matmul(
                        ps[:], ident[:], zxh,
                        start=True, stop=False, skip_group_check=True,
                    )

                def pair(half, m, kk, stop=False):
                    a, q = m // 4, (m % 4) - 2 * half
                    dst = pss[half][:, a * 16 + q * 8: a * 16 + q * 8 + 8]
                    return nc.tensor.matmul(
                        dst,
                        wr_sb[:, (kk * M + m) * P:(kk * M + m + 1) * P],
                        h_prev[kk // 2][:, (kk % 2) * B_LOC:
                                        (kk % 2 + 1) * B_LOC],
                        start=False, stop=stop, skip_group_check=True,
                    )

                # blocks: (half, [(m, kk)...], stop_at_end)
                b_a01 = [(0, m, kk) for kk in (0, 1) for m in HALF_MS[0]]
                b_b01 = [(1, m, kk) for kk in (0, 1) for m in HALF_MS[1]]
                b_a23 = [(0, m, kk) for kk in (2, 3) for m in HALF_MS[0]]
                b_b23 = [(1, m, kk) for kk in (2, 3) for m in HALF_MS[1]]
                sched = (b_a01 + b_b01[:12] + b_a23 + b_b01[12:] + b_b23)
                stop_mms = [None, None]
                prev_p = None
                for idx, (half, m, kk) in enumerate(sched):
                    is_stop = (idx == 43 and half == 0) or (idx == 63)
                    i_mm = pair(half, m, kk, stop=is_stop)
                    if is_stop:
                        stop_mms[half] = i_mm
                    if prev_p is not None:
                        add_dep_helper(i_mm.ins, prev_p.ins,
                                       reason="LP pair order")
                    prev_p = i_mm
                # --- ACT: gate sigmoids; psum holds 256*z (fp8 scaling), the
                # ACT input-scale folds the descale in for free ---
                gts, scs = [], []
                for half in range(2):
                    gt = gate_pool.tile([P, HB], f32, name=f"gt{half}",
                                        tag=f"gt{half}")
                    gts.append(gt)
                    nc.scalar.activation(gt[:], pss[half][:], AF.Sigmoid)
                # --- DVE: c update per half ---
                # (half0's chain ops are forced ahead of half1's in the DVE
                # queue so half1's readiness doesn't delay AFF0)
                last = t == time_steps - 1
                prev_aff = None
                for half in range(2):
                    gt = gts[half]
                    v = tmp_pool.tile([P, 2 * B_LOC], f32, name=f"v{half}",
                                      tag=f"v{half}")
                    # v = (s_g - 0.5) * s_i
                    i_v = nc.vector.scalar_tensor_tensor(
                        v[:], gt[:, 48:64], -0.5, gt[:, 0:16],
                        ALU.add, ALU.mult,
                    )
                    t1 = tmp_pool.tile([P, 2 * B_LOC], f32, name=f"t1{half}",
                                       tag=f"t1{half}")
                    i_t1 = nc.vector.tensor_mul(t1[:], gt[:, 16:32], cs[half][:])
                    # C' = 4v + t1   (C = 2c)
                    i_aff = nc.vector.affine_then_add(cs[half][:], v[:], t1[:],
                                                      4.0, 0.0)
                    if prev_aff is not None:
                        # keep ALL of chain1's DVE ops behind chain0's AFF so
                        # they can't bubble the DVE queue ahead of it
                        add_dep_helper(i_v.ins, prev_aff.ins,
                                       reason="chain1 DVE after chain0 AFF")
                        add_dep_helper(i_t1.ins, prev_aff.ins,
                                       reason="chain1 DVE after chain0 AFF")
                    prev_aff = i_aff
                i_sc1 = None
                for half in range(2):
                    sc = tmp_pool.tile([P, 2 * B_LOC], f32, name=f"sc{half}",
                                       tag=f"sc{half}")
                    scs.append(sc)
                    i_sc1 = nc.scalar.activation(sc[:], cs[half][:], AF.Sigmoid)
                for half in range(2):
                    # h' = (sc - 0.5) * s_o   (= true h / 2)
                    dst = (hf[:, half * 2 * B_LOC:(half + 1) * 2 * B_LOC]
                           if last else hs[qq][half][:])
                    nc.vector.scalar_tensor_tensor(
                        dst, scs[half][:], -0.5, gts[half][:, 32:48],
                        ALU.add, ALU.mult,
                    )
                # --- phase-A interleave (pinned into this step's tail) ---
                c_mm = t // TC + 1
                j = t % TC
                if c_mm < NCH:
                    chunk_mm(c_mm, j, pin_mm=stop_mms[1], pin_act=i_sc1)
                c_in = t // TC + 2
                if c_in < NCH:
                    chunk_input(c_in, j)
                # --- dummy matmuls in the tail window: keep the PE HAM
                # activity monitor busy so the array stays at 2.4 GHz ---
                prev_d = stop_mms[1]
                for _dk in range(N_DUMMY):
                    i_d = nc.tensor.matmul(
                        dps[:], ident[:], ident[:, 0:HB],
                        start=True, stop=True, skip_group_check=True,
                    )
                    add_dep_helper(i_d.ins, prev_d.ins, reason="warm fill")
                    prev_d = i_d

            # ---------------- output ----------------
            nc.vector.tensor_add(hf[:], hf[:], hf[:])  # x2: h was stored /2
            for kk in range(KU):
                nc.sync.dma_start(
                    out_h.ap()[:, kk * P:(kk + 1) * P].rearrange("b p -> p b"),
                    hf[:, kk * B_LOC:(kk + 1) * B_LOC],
                )

    nc.compile()
    return nc


def _get_nc(time_steps=T):
    key = time_steps
    if key not in _CACHE:
        _CACHE[key] = _build(time_steps)
    return _CACHE[key]


def _make_in_maps(inputs):
    x = np.ascontiguousarray(np.asarray(inputs["x"], dtype=np.float32))
    Wk = np.ascontiguousarray(np.asarray(inputs["Wk"], dtype=np.float32))
    Wr = np.ascontiguousarray(np.asarray(inputs["Wr"], dtype=np.float32))
    b = np.ascontiguousarray(np.asarray(inputs["b"], dtype=np.float32))
    return [
        {
            "x": x[c * B_LOC:(c + 1) * B_LOC],
            "Wk": Wk,
            "Wr": Wr,
            "b": b,
        }
        for c in range(N_CORES)
    ]


def _gather_output(res):
    return np.concatenate(
        [res.results[c]["h_last"] for c in range(N_CORES)], axis=0
    )


def kernel(x, Wk, Wr, b):
    from concourse import bass_utils

    nc = _get_nc(T)
    in_maps = _make_in_maps({"x": x, "Wk": Wk, "Wr": Wr, "b": b})
    res = bass_utils.run_bass_kernel_spmd(nc, in_maps, core_ids=list(range(N_CORES)))
    return _gather_output(res)
